# revision 18
# baseline (speedup 1.0000x reference)
"""AGCN block (LayerNorm -> adaptive adjacency w/ top-k -> BatchNorm -> Chebyshev
graph conv) on 8 TRN2 NeuronCores, pure data-parallel over batch.

Per core (8 samples):
  - LayerNorm stats via bn_stats + PE cross-partition combine; apply on ACT (bf16).
  - BatchNorm batch stats: tiny (128,4) AllReduce overlapped with adjacency work.
  - xp / scores / adjacency / Chebyshev matmuls in bf16 on PE (f32 PSUM accum);
    xp pairs two samples per matmul to halve LDWEIGHTS+instruction count.
  - All 128x128 transposes on the (otherwise idle) DMA engines via the xbar
    (dma_start_transpose, SBUF->SBUF bf16) instead of TensorE+PSUM+copy.
  - top-51 row threshold: batched bisection on is_ge counts, split between
    DVE (tensor_scalar accum) and ACT (Sign accum) with per-column thresholds.
  - Chebyshev K=3 with D^-1/2 folded in as per-partition scales.
"""

import os
import sys

import numpy as np

for _p in ("/opt/trn_rl_repo", "/opt/pypackages"):
    if _p not in sys.path:
        sys.path.append(_p)

import concourse.bass as bass
import concourse.mybir as mybir
from concourse import bacc
from concourse.bass_utils import run_bass_kernel_spmd
from concourse.tile import TileContext

F32 = mybir.dt.float32
BF16 = mybir.dt.bfloat16
AF = mybir.ActivationFunctionType
OP = mybir.AluOpType

N_CORES = 8
B, N, T = 64, 256, 512
SPC = B // N_CORES          # samples per core
NT = N // 128               # node tiles (2)
TT = T // 128               # t tiles (4)
N_MAX = N // 5              # 51
BISECT_ITERS = 12
N_ACT_TILES = 3             # per 8-tile bisect group, how many counts go to ACT
EPS_NORM = 1e-5
EPS_DEG = 1e-10

LAST_RESULT = None


def _build(ones_ln_w, zeros_ln_b, ones_bn_g, zeros_bn_b, zeros_li_b, zeros_cheb_b):
    nc = bacc.Bacc("TRN2", target_bir_lowering=False, num_devices=N_CORES)

    x_ext = nc.declare_dram_parameter("x", [SPC, N, T], F32, isOutput=False)
    dis_ext = nc.declare_dram_parameter("dis_adj", [N, N], F32, isOutput=False)
    lnw_ext = nc.declare_dram_parameter("ln_w", [N, T], F32, isOutput=False)
    lnb_ext = nc.declare_dram_parameter("ln_b", [N, T], F32, isOutput=False)
    bng_ext = nc.declare_dram_parameter("bn_g", [N], F32, isOutput=False)
    bnb_ext = nc.declare_dram_parameter("bn_b", [N], F32, isOutput=False)
    liw_ext = nc.declare_dram_parameter("li_w", [T, T], F32, isOutput=False)
    lib_ext = nc.declare_dram_parameter("li_b", [T], F32, isOutput=False)
    cw_ext = nc.declare_dram_parameter("cheb_w", [3, T, T], F32, isOutput=False)
    cb_ext = nc.declare_dram_parameter("cheb_b", [T], F32, isOutput=False)
    out_ext = nc.declare_dram_parameter("out", [SPC, N, T], F32, isOutput=True)

    from contextlib import ExitStack
    with TileContext(nc) as tc, ExitStack() as ctx:
        consts = ctx.enter_context(tc.tile_pool(name="consts", bufs=1))
        persist = ctx.enter_context(tc.tile_pool(name="persist", bufs=1))
        work = ctx.enter_context(tc.tile_pool(name="work", bufs=2))
        small = ctx.enter_context(tc.tile_pool(name="small", bufs=2))
        dram = ctx.enter_context(tc.tile_pool(name="dram", bufs=1, space="DRAM"))
        ps_mm = ctx.enter_context(tc.tile_pool(name="ps_mm", bufs=3, space="PSUM"))
        ps_sc = ctx.enter_context(tc.tile_pool(name="ps_sc", bufs=3, space="PSUM"))
        ps_ln = ctx.enter_context(tc.tile_pool(name="ps_ln", bufs=1, space="PSUM"))

        # ---------------- one-time constants ----------------
        ones_col_f32 = consts.tile([128, 1], F32)
        nc.vector.memset(ones_col_f32, 1.0)
        ones_row_f32 = consts.tile([1, 128], F32)
        nc.vector.memset(ones_row_f32, 1.0)

        cbr = ones_row_bf16 = None
        if not zeros_cheb_b:
            ones_row_bf16 = consts.tile([1, 128], BF16)
            nc.vector.memset(ones_row_bf16, 1.0)
            cbr_f32 = consts.tile([1, T], F32)
            nc.gpsimd.dma_start(out=cbr_f32,
                                in_=cb_ext[:].rearrange("(a f) -> a f", a=1))
            cbr = consts.tile([1, T], BF16)
            nc.vector.tensor_copy(cbr, cbr_f32)

        libc = None
        if not zeros_li_b:
            libc = consts.tile([128, TT], F32)
            nc.gpsimd.dma_start(out=libc,
                                in_=lib_ext[:].rearrange("(t p) -> p t", p=128))

        bngc = bnbc = None
        if not ones_bn_g:
            bngc = consts.tile([128, NT], F32)
            nc.gpsimd.dma_start(out=bngc,
                                in_=bng_ext[:].rearrange("(t p) -> p t", p=128))
        if not zeros_bn_b:
            bnbc = consts.tile([128, NT], F32)
            nc.gpsimd.dma_start(out=bnbc,
                                in_=bnb_ext[:].rearrange("(t p) -> p t", p=128))

        DIS = consts.tile([128, NT, N], BF16)
        dstage = work.tile([128, NT, N], F32, tag="dstage", bufs=1)
        nc.gpsimd.dma_start(out=dstage,
                            in_=dis_ext.rearrange("(t p) m -> p t m", p=128))
        nc.vector.tensor_copy(DIS, dstage)

        LNW = LNB = None
        if not (ones_ln_w and zeros_ln_b):
            LNW = consts.tile([128, NT, T], BF16)
            LNB = consts.tile([128, NT, T], BF16)
            wst = work.tile([128, NT, T], F32, tag="lnwst", bufs=1)
            nc.gpsimd.dma_start(out=wst,
                                in_=lnw_ext.rearrange("(t p) f -> p t f", p=128))
            nc.scalar.copy(LNW, wst)
            bst = work.tile([128, NT, T], F32, tag="lnbst", bufs=1)
            nc.gpsimd.dma_start(out=bst,
                                in_=lnb_ext.rearrange("(t p) f -> p t f", p=128))
            nc.scalar.copy(LNB, bst)

        CW = consts.tile([128, 3, TT, T], BF16)
        for k in range(3):
            cst = work.tile([128, TT, T], F32, tag=f"cwst{k}", bufs=1,
                            name=f"cwst{k}")
            nc.gpsimd.dma_start(out=cst,
                                in_=cw_ext[k].rearrange("(t p) f -> p t f", p=128))
            nc.scalar.copy(CW[:, k], cst)

        # li_w^T bf16 (t-major) via DMA xbar transpose
        LWS = consts.tile([128, TT, T], BF16)   # f-major staging
        lst = work.tile([128, TT, T], F32, tag="lwst", bufs=1)
        nc.gpsimd.dma_start(out=lst,
                            in_=liw_ext.rearrange("(t p) f -> p t f", p=128))
        nc.vector.tensor_copy(LWS, lst)
        LWT = consts.tile([128, TT, T], BF16)   # t-major
        for tt in range(TT):
            for ft in range(TT):
                nc.sync.dma_start_transpose(
                    out=LWT[:, tt, ft * 128:(ft + 1) * 128],
                    in_=LWS[:, ft, tt * 128:(tt + 1) * 128])

        # bisection per-column ge-thresholds: DVE cols count>=50.5,
        # ACT cols signsum >= 2*51-256-0.5
        TH = consts.tile([128, SPC * NT], F32)
        nc.vector.memset(TH, float(N_MAX) - 0.5)

        # ---------------- persistent state ----------------
        XLN = persist.tile([128, SPC, NT, T], BF16)
        S = persist.tile([128, SPC, NT, N], BF16)
        BNS = persist.tile([128, NT, SPC], F32)
        BNQ = persist.tile([128, NT, SPC], F32)
        SCS = persist.tile([128, SPC * NT], F32)
        SCQ = persist.tile([128, SPC * NT], F32)
        M8 = persist.tile([128, SPC * NT, 8], F32)
        LO = persist.tile([128, SPC * NT], F32)
        CNT = persist.tile([128, SPC * NT, N], BF16)
        NEGMID = persist.tile([128, SPC * NT], F32)
        DEG = persist.tile([128, NT, SPC], F32)
        DINV = persist.tile([128, NT, SPC], F32)
        D2 = persist.tile([128, NT, SPC], F32)
        ALPHA = persist.tile([128, NT], F32)
        BETAF = persist.tile([128, NT, T], BF16)
        AT_all = persist.tile([128, SPC, NT, N], BF16)

        def tix(s, nt):
            return s * NT + nt

        act_tile = {}
        for g in range(2):
            for i, s in enumerate(range(g * 4, g * 4 + 4)):
                for nt in range(NT):
                    j = tix(s, nt)
                    act_tile[j] = (i * NT + nt) >= (8 - N_ACT_TILES)
                    if act_tile[j]:
                        nc.vector.memset(TH[:, j:j + 1], 2.0 * N_MAX - N - 0.5)

        # ---------------- per-pair phase A ----------------
        def phase_a_pair(p):
            sa, sb = 2 * p, 2 * p + 1
            Xp = work.tile([128, 2, NT, T], F32, tag="xraw", name=f"x{p}", bufs=3)
            lns_all = []
            for i, s in enumerate((sa, sb)):
                nc.gpsimd.dma_start(out=Xp[:, i],
                                    in_=x_ext[s].rearrange("(t p) f -> p t f", p=128))
                X = Xp[:, i]
                st6 = small.tile([128, NT, 6], F32, tag="st6", name=f"st6_{s}")
                mv = small.tile([128, NT, 2], F32, tag="mv", name=f"mv{s}")
                par = small.tile([128, 2 * NT], F32, tag="par", name=f"par{s}")
                for nt in range(NT):
                    nc.vector.bn_stats(st6[:, nt], X[:, nt])
                    nc.vector.bn_aggr(mv[:, nt], st6[:, nt])
                    nc.vector.tensor_scalar_mul(par[:, nt:nt + 1], mv[:, nt, 0:1],
                                                float(T))
                    nc.vector.scalar_tensor_tensor(
                        par[:, NT + nt:NT + nt + 1], mv[:, nt, 0:1], mv[:, nt, 0:1],
                        mv[:, nt, 1:2], op0=OP.mult, op1=OP.add)
                    nc.vector.tensor_scalar_mul(
                        par[:, NT + nt:NT + nt + 1], par[:, NT + nt:NT + nt + 1],
                        float(T))
                psl = ps_ln.tile([1, 4], F32, tag="lna", name=f"psl{s}")
                nc.tensor.matmul(psl, ones_col_f32, par, start=True, stop=True)
                a = small.tile([1, 6], F32, tag="sc2", name=f"sc2_{s}")
                inv_cnt = 1.0 / float(N * T)
                nc.vector.tensor_copy(a[:, 2:6], psl[:, 0:4])
                nc.vector.tensor_add(a[:, 0:1], a[:, 2:3], a[:, 3:4])
                nc.vector.tensor_add(a[:, 1:2], a[:, 4:5], a[:, 5:6])
                nc.vector.tensor_scalar_mul(a[:, 0:1], a[:, 0:1], inv_cnt)
                nc.vector.tensor_scalar_mul(a[:, 1:2], a[:, 1:2], inv_cnt)
                nc.vector.scalar_tensor_tensor(
                    a[:, 3:4], a[:, 0:1], a[:, 0:1], a[:, 1:2],
                    op0=OP.mult, op1=OP.subtract)
                nc.vector.tensor_scalar(a[:, 3:4], a[:, 3:4], -1.0, EPS_NORM,
                                        op0=OP.mult, op1=OP.add)
                nc.vector.reciprocal(a[:, 3:4], a[:, 3:4])
                nc.scalar.sqrt(a[:, 3:4], a[:, 3:4])
                nc.vector.tensor_scalar(a[:, 2:3], a[:, 0:1], a[:, 3:4], -1.0,
                                        op0=OP.mult, op1=OP.mult)
                psb = ps_ln.tile([128, 2], F32, tag="lnb", name=f"psb{s}")
                nc.tensor.matmul(psb, ones_row_f32, a[:, 2:4], start=True, stop=True)
                lns = small.tile([128, 2], F32, tag="lns", name=f"lns{s}")
                nc.vector.tensor_copy(lns, psb)
                lns_all.append(lns)

            sqs = work.tile([128, NT, T], BF16, tag="sqs", name=f"sqs{p}")
            for i, s in enumerate((sa, sb)):
                lns = lns_all[i]
                for nt in range(NT):
                    if LNW is None:
                        nc.scalar.activation(XLN[:, s, nt], Xp[:, i, nt], AF.Identity,
                                             bias=lns[:, 0:1], scale=lns[:, 1:2],
                                             accum_out=BNS[:, nt, s:s + 1])
                    else:
                        xact = work.tile([128, T], BF16, tag="xact",
                                         name=f"xact{s}_{nt}")
                        nc.scalar.activation(xact, Xp[:, i, nt], AF.Identity,
                                             bias=lns[:, 0:1], scale=lns[:, 1:2])
                        tmp = work.tile([128, T], BF16, tag="xtmp",
                                        name=f"xtmp{s}_{nt}")
                        nc.vector.scalar_tensor_tensor(
                            tmp, xact, 1.0, LNW[:, nt], op0=OP.bypass, op1=OP.mult)
                        nc.vector.scalar_tensor_tensor(
                            XLN[:, s, nt], tmp, 1.0, LNB[:, nt],
                            op0=OP.bypass, op1=OP.add,
                            accum_out=BNS[:, nt, s:s + 1])
                    nc.scalar.activation(sqs[:, nt], XLN[:, s, nt], AF.Square,
                                         accum_out=BNQ[:, nt, s:s + 1])

            # transpose x_ln -> Y pair (t-major), via DMA xbar
            Y = work.tile([128, TT, 2 * N], BF16, tag="y", name=f"y{p}")
            for i, s in enumerate((sa, sb)):
                for tt in range(TT):
                    for nt in range(NT):
                        nc.sync.dma_start_transpose(
                            out=Y[:, tt, i * N + nt * 128:i * N + (nt + 1) * 128],
                            in_=XLN[:, s, nt, tt * 128:(tt + 1) * 128])

            # xp^T pair (f-major): lhsT = li_w^T chunk, rhs = Y pair
            XPT = work.tile([128, TT, 2 * N], BF16, tag="xpt", name=f"xpt{p}")
            for ft in range(TT):
                ps = ps_mm.tile([128, T], F32, tag="mm", name=f"xps{p}_{ft}")
                for kt in range(TT):
                    nc.tensor.matmul(ps, LWT[:, kt, ft * 128:(ft + 1) * 128],
                                     Y[:, kt], start=(kt == 0), stop=(kt == TT - 1))
                if libc is None:
                    nc.scalar.activation(XPT[:, ft], ps, AF.Identity)
                else:
                    nc.scalar.activation(XPT[:, ft], ps, AF.Identity,
                                         bias=libc[:, ft:ft + 1])

            # scores per sample
            for i, s in enumerate((sa, sb)):
                for nt in range(NT):
                    ps = ps_sc.tile([128, N], F32, tag="sc", name=f"scps{s}_{nt}")
                    for kt in range(TT):
                        nc.tensor.matmul(
                            ps, XPT[:, kt, i * N + nt * 128:i * N + (nt + 1) * 128],
                            XPT[:, kt, i * N:(i + 1) * N],
                            start=(kt == 0), stop=(kt == TT - 1))
                    j = tix(s, nt)
                    nc.vector.tensor_scalar(S[:, s, nt], ps, 1.0, 0.0, op0=OP.mult,
                                            op1=OP.add, accum_out=SCS[:, j:j + 1])
                    nc.vector.max(M8[:, j], S[:, s, nt])
                    sq2 = work.tile([128, N], BF16, tag="sq2", name=f"sq2_{s}_{nt}")
                    nc.scalar.activation(sq2, S[:, s, nt], AF.Square,
                                         accum_out=SCQ[:, j:j + 1])

        # ---------------- batched bisection over a 4-sample group ----------------
        def bisect(group):
            g0 = group[0]
            c0, c1 = tix(g0, 0), tix(group[-1], NT - 1) + 1
            w = c1 - c0
            MU = small.tile([128, w], F32, tag="bmu", name=f"mu{g0}")
            E2 = small.tile([128, w], F32, tag="be2", name=f"e2_{g0}")
            SD = small.tile([128, w], F32, tag="bsd", name=f"sd{g0}")
            HI = small.tile([128, w], F32, tag="bhi", name=f"hi{g0}")
            WD = small.tile([128, w], F32, tag="bwd", name=f"wd{g0}")
            MID = small.tile([128, w], F32, tag="bmid", name=f"mid{g0}")
            C = small.tile([128, w], F32, tag="bc", name=f"c{g0}")
            GE = small.tile([128, w], F32, tag="bge", name=f"ge{g0}")
            inv_n = 1.0 / float(N)
            nc.vector.tensor_scalar_mul(MU, SCS[:, c0:c1], inv_n)
            nc.vector.tensor_scalar_mul(E2, SCQ[:, c0:c1], inv_n)
            nc.vector.tensor_mul(SD, MU, MU)
            nc.vector.tensor_sub(SD, E2, SD)
            nc.vector.tensor_scalar_max(SD, SD, 1e-12)
            nc.scalar.sqrt(SD, SD)
            nc.vector.scalar_tensor_tensor(LO[:, c0:c1], SD, -10.0, MU,
                                           op0=OP.mult, op1=OP.add)
            nc.vector.tensor_copy(HI, M8[:, c0:c1, 7:8].rearrange("p a b -> p (a b)"))
            nc.vector.tensor_sub(WD, HI, LO[:, c0:c1])
            for it in range(BISECT_ITERS):
                nc.vector.tensor_scalar_mul(WD, WD, 0.5)
                nc.vector.tensor_add(MID, LO[:, c0:c1], WD)
                nc.vector.tensor_scalar_mul(NEGMID[:, c0:c1], MID, -1.0)
                for j in range(c0, c1):
                    if act_tile[j]:
                        nc.scalar.activation(
                            CNT[:, j], S[:, j // NT, j % NT], AF.Sign,
                            bias=NEGMID[:, j:j + 1],
                            accum_out=C[:, j - c0:j - c0 + 1])
                    else:
                        nc.vector.tensor_scalar(
                            CNT[:, j], S[:, j // NT, j % NT],
                            MID[:, j - c0:j - c0 + 1],
                            0.0, op0=OP.is_ge, op1=OP.add,
                            accum_out=C[:, j - c0:j - c0 + 1])
                nc.vector.tensor_tensor(GE, C, TH[:, c0:c1], op=OP.is_ge)
                nc.vector.tensor_mul(GE, GE, WD)
                nc.vector.tensor_add(LO[:, c0:c1], LO[:, c0:c1], GE)

        # ---------------- per-sample mask/adjacency ----------------
        def phase_mask(s):
            A = work.tile([128, NT, N], BF16, tag="a", name=f"a{s}")
            for nt in range(NT):
                j = tix(s, nt)
                msk = work.tile([128, N], BF16, tag="msk", name=f"msk{s}_{nt}")
                nc.vector.scalar_tensor_tensor(
                    msk, S[:, s, nt], LO[:, j:j + 1], S[:, s, nt],
                    op0=OP.is_ge, op1=OP.mult)
                aw = work.tile([128, N], BF16, tag="aw", name=f"aw{s}_{nt}")
                nc.vector.tensor_add(aw, msk, DIS[:, nt])
                nc.scalar.activation(A[:, nt], aw, AF.Relu,
                                     accum_out=DEG[:, nt, s:s + 1])
            for mt in range(NT):
                for nt in range(NT):
                    nc.sync.dma_start_transpose(
                        out=AT_all[:, s, mt, nt * 128:(nt + 1) * 128],
                        in_=A[:, nt, mt * 128:(mt + 1) * 128])

        def dinv_group(group):
            s0, s1 = group[0], group[-1] + 1
            df = DEG[:, :, s0:s1]
            vf = DINV[:, :, s0:s1]
            d2f = D2[:, :, s0:s1]
            nc.vector.tensor_scalar_add(vf, df, EPS_DEG)
            nc.vector.reciprocal(vf, vf)
            nc.scalar.sqrt(vf, vf)
            nc.vector.tensor_scalar_mul(d2f, vf, 2.0)

        # ---------------- per-sample Chebyshev + output ----------------
        def phase_c(s):
            xbn = work.tile([128, NT, T], BF16, tag="xbn", name=f"xbn{s}")
            u = work.tile([128, NT, T], BF16, tag="u", name=f"u{s}")
            for nt in range(NT):
                nc.vector.scalar_tensor_tensor(
                    xbn[:, nt], XLN[:, s, nt], ALPHA[:, nt:nt + 1], BETAF[:, nt],
                    op0=OP.mult, op1=OP.add)
                nc.vector.tensor_scalar_mul(u[:, nt], xbn[:, nt], DINV[:, nt, s:s + 1])

            Tx1 = work.tile([128, NT, T], BF16, tag="tx1", name=f"tx1_{s}")
            u2 = work.tile([128, NT, T], BF16, tag="u2", name=f"u2_{s}")
            for nt in range(NT):
                ps = ps_mm.tile([128, T], F32, tag="mm", name=f"w1ps{s}_{nt}")
                for kt in range(NT):
                    nc.tensor.matmul(ps, AT_all[:, s, kt, nt * 128:(nt + 1) * 128],
                                     u[:, kt], start=(kt == 0), stop=(kt == NT - 1))
                nc.scalar.activation(Tx1[:, nt], ps, AF.Copy,
                                     scale=DINV[:, nt, s:s + 1])
                nc.vector.tensor_scalar_mul(u2[:, nt], Tx1[:, nt],
                                            DINV[:, nt, s:s + 1])
            Tx2 = work.tile([128, NT, T], BF16, tag="tx2", name=f"tx2_{s}")
            for nt in range(NT):
                ps = ps_mm.tile([128, T], F32, tag="mm", name=f"w2ps{s}_{nt}")
                for kt in range(NT):
                    nc.tensor.matmul(ps, AT_all[:, s, kt, nt * 128:(nt + 1) * 128],
                                     u2[:, kt], start=(kt == 0), stop=(kt == NT - 1))
                t2t = work.tile([128, T], BF16, tag="t2t", name=f"t2t{s}_{nt}")
                nc.scalar.activation(t2t, ps, AF.Copy, scale=D2[:, nt, s:s + 1])
                nc.vector.tensor_sub(Tx2[:, nt], t2t, xbn[:, nt])

            TXT = work.tile([128, 3, TT, N], BF16, tag="txt", name=f"txt{s}")
            for k, src in enumerate((xbn, Tx1, Tx2)):
                for tt in range(TT):
                    for nt in range(NT):
                        nc.sync.dma_start_transpose(
                            out=TXT[:, k, tt, nt * 128:(nt + 1) * 128],
                            in_=src[:, nt, tt * 128:(tt + 1) * 128])

            OUTS = work.tile([128, NT, T], F32, tag="outs", name=f"outs{s}")
            for nt in range(NT):
                ps = ps_mm.tile([128, T], F32, tag="mm", name=f"ops{s}_{nt}")
                n_mm = 3 * TT + (0 if cbr is None else 1)
                i_mm = 0
                for k in range(3):
                    for kt in range(TT):
                        nc.tensor.matmul(ps, TXT[:, k, kt, nt * 128:(nt + 1) * 128],
                                         CW[:, k, kt], start=(i_mm == 0),
                                         stop=(i_mm == n_mm - 1))
                        i_mm += 1
                if cbr is not None:
                    nc.tensor.matmul(ps, ones_row_bf16, cbr, start=False, stop=True)
                nc.scalar.activation(OUTS[:, nt], ps, AF.Relu)
            nc.gpsimd.dma_start(out=out_ext[s].rearrange("(t p) f -> p t f", p=128),
                                in_=OUTS)

        # ---------------- emit program ----------------
        for p in range(2):
            phase_a_pair(p)
        bisect([0, 1, 2, 3])
        for p in range(2, 4):
            phase_a_pair(p)

        # BatchNorm all-reduce of (sum, sumsq) per node
        stage = small.tile([128, 2 * NT], F32, tag="bnstage")
        nc.vector.tensor_reduce(stage[:, 0:NT], BNS, mybir.AxisListType.X, OP.add)
        nc.vector.tensor_reduce(stage[:, NT:2 * NT], BNQ, mybir.AxisListType.X, OP.add)
        bn_in = dram.tile([128, 2 * NT], F32)
        bn_out = dram.tile([128, 2 * NT], F32, addr_space="Shared")
        nc.gpsimd.dma_start(out=bn_in[:], in_=stage[:])
        nc.gpsimd.collective_compute(
            "AllReduce", OP.add, replica_groups=[list(range(N_CORES))],
            ins=[bn_in.opt()], outs=[bn_out.opt()])
        arr = small.tile([128, 2 * NT], F32, tag="bnarr")
        nc.gpsimd.dma_start(out=arr, in_=bn_out[:])

        for s in range(4):
            phase_mask(s)
        dinv_group([0, 1, 2, 3])
        bisect([4, 5, 6, 7])

        # batchnorm affine from all-reduced stats
        inv_bt = 1.0 / float(B * T)
        BM = small.tile([128, NT], F32, tag="bm")
        RSQ = small.tile([128, NT], F32, tag="rsq")
        BETA = small.tile([128, NT], F32, tag="beta")
        nc.vector.tensor_scalar_mul(BM, arr[:, 0:NT], inv_bt)
        nc.vector.tensor_scalar_mul(RSQ, arr[:, NT:2 * NT], inv_bt)
        tmpv = small.tile([128, NT], F32, tag="tmpv")
        nc.vector.tensor_mul(tmpv, BM, BM)
        nc.vector.tensor_sub(RSQ, RSQ, tmpv)
        nc.vector.tensor_scalar_add(RSQ, RSQ, EPS_NORM)
        nc.vector.reciprocal(RSQ, RSQ)
        nc.scalar.sqrt(RSQ, RSQ)
        if bngc is None:
            nc.vector.tensor_copy(ALPHA, RSQ)
        else:
            nc.vector.tensor_mul(ALPHA, RSQ, bngc)
        nega = small.tile([128, NT], F32, tag="nega")
        nc.vector.tensor_scalar_mul(nega, ALPHA, -1.0)
        for nt in range(NT):
            if bnbc is None:
                nc.vector.tensor_mul(BETA[:, nt:nt + 1], BM[:, nt:nt + 1],
                                     nega[:, nt:nt + 1])
            else:
                nc.vector.scalar_tensor_tensor(
                    BETA[:, nt:nt + 1], BM[:, nt:nt + 1], nega[:, nt:nt + 1],
                    bnbc[:, nt:nt + 1], op0=OP.mult, op1=OP.add)
            nc.vector.tensor_copy(BETAF[:, nt],
                                  BETA[:, nt:nt + 1].to_broadcast([128, T]))

        for s in range(4):
            phase_c(s)
        for s in range(4, SPC):
            phase_mask(s)
        dinv_group([4, 5, 6, 7])
        for s in range(4, SPC):
            phase_c(s)

    nc.finalize()
    return nc


_BUILD_CACHE = {}


def kernel(**inputs):
    global LAST_RESULT
    x = np.ascontiguousarray(np.asarray(inputs["x"], dtype=np.float32))
    flags = (
        bool(np.all(inputs["ln_w"] == 1.0)), bool(np.all(inputs["ln_b"] == 0.0)),
        bool(np.all(inputs["bn_g"] == 1.0)), bool(np.all(inputs["bn_b"] == 0.0)),
        bool(np.all(inputs["li_b"] == 0.0)), bool(np.all(inputs["cheb_b"] == 0.0)),
    )
    if flags not in _BUILD_CACHE:
        _BUILD_CACHE[flags] = _build(*flags)
    nc = _BUILD_CACHE[flags]

    common = {k: np.ascontiguousarray(np.asarray(inputs[k], dtype=np.float32))
              for k in ("dis_adj", "ln_w", "ln_b", "bn_g", "bn_b", "li_w", "li_b",
                        "cheb_w", "cheb_b")}
    in_maps = []
    for c in range(N_CORES):
        m = dict(common)
        m["x"] = x[c * SPC:(c + 1) * SPC]
        in_maps.append(m)

    res = run_bass_kernel_spmd(
        nc, in_maps, list(range(N_CORES)),
        trace=bool(int(os.environ.get("KERNEL_TRACE", "0"))),
    )
    LAST_RESULT = res
    out = np.concatenate([np.asarray(res.results[c]["out"]) for c in range(N_CORES)],
                         axis=0)
    return out
